# revision 18
# baseline (speedup 1.0000x reference)
"""Trainium2 Bass kernel for nn_Attention_85856396247857.

16-head causal attention with rotary embeddings, fp32 in/out, x:[2,2048,2048].

Sharding (8 cores): core c = (b, g) with b = c // 4 (batch), g = c % 4
(head group). Each core handles batch b and heads 4g..4g+3 (tensor
parallel: Wq/Wk/Wv column-sliced by head, Wo row-sliced; the row-parallel
output partials are summed on the host).

v2: all operands bf16 (PSUM accumulation stays f32) — bf16 moving operands
stream ~2 cols/cycle on TRN2 (measured ~130ns per 512-col LDW+MM pair), so
the projection phases run at ~2x the f32 rate. Single fused pass over x
computes Q^T/K^T (rotary fused, rotate_half via a 128x128 permutation
matmul) and V; flash-style attention in S^T layout with paired-block exp
instructions and causal column-offset trimming on all diagonal blocks;
output projection interleaved per 512-query tile.

v3 (measured-bottleneck round): ps_o/ps_r double-buffered so consecutive
heads' flush chains never wait on the previous head's normalize;
reciprocal_approx_fast (~5x cheaper on DVE, 18-bit accurate — plenty for
softmax denominators); C-phase drains two 512-col PSUM chunks per copy
instruction, alternating DVE/ACT; output DMAs ride the Activation HWDGE
queue so they never queue behind x/weight input traffic on the SP queue.
HW phase split (R=17 amplification): A ~106us (PE-bound at the measured
~130ns/512-col-MM stream rate), B ~69us (ACT exp-bound), C ~70us.
Run-to-run variance across processes is +-40us (HBM buffer placement
lottery in PJRT allocation) — config deltas below ~20us are noise.

Numerics: bf16 rounding gives rel err ~3.5e-3 vs the f32 reference
(tolerance 2e-2); exp needs no max-subtraction since |S| < 6.

`phases` / `repeat` exist for benchmarking (differential phase timing and
in-NEFF amplification); production uses the defaults.
"""

import os
import sys

import numpy as np

for _p in ("/opt/trn_rl_repo",):
    if _p not in sys.path and os.path.isdir(_p):
        sys.path.insert(0, _p)

import ml_dtypes  # noqa: E402

import concourse.bass as bass  # noqa: E402
import concourse.mybir as mybir  # noqa: E402
import concourse.tile as tile  # noqa: E402
from concourse import bacc  # noqa: E402
from concourse.bass_utils import run_bass_kernel_spmd  # noqa: E402

F32 = mybir.dt.float32
BF16 = mybir.dt.bfloat16
NPBF = ml_dtypes.bfloat16
EXP = mybir.ActivationFunctionType.Exp

# Problem shape (hardcoded per contract)
B, N, D = 2, 2048, 2048
H, DH = 16, 128
NCORES = 8
GROUPS = 4              # head groups (tensor parallel)
HPC = H // GROUPS       # heads per core = 4
INNER_C = HPC * DH      # per-core inner dim = 512

KSL = D // 128          # 16 contraction slices
ST = 512                # seq columns per tile / query-block granularity
NIT = N // ST           # 4
NJB = N // 128          # 16 key blocks

_CACHE = {}
LAST_RESULTS = None
PHASE_MARKS = []  # (instruction-id watermark, label) per build, for simbench

# Tunables for A/B benchmarking of scheduling variants (bench_variants.py).
CONFIG = {
    "c_copy": "alt",      # "alt" (DVE/ACT alternate) | "dve" (all DVE)
    "out_queue": "act",   # "sp" | "act" | "both" — queue(s) for C's out DMAs
    "w_queue": "sp",      # "sp" | "act" — queue for wk/wv/wo weight loads
    "osb_bufs": 2,        # out-staging SBUF double-buffer depth
    "ps_out_bufs": 4,     # C-phase PSUM buffer depth
    "c_pair": 1,          # drain two 512-col PSUM chunks per copy inst
}


def _mark(nc, label):
    try:
        PHASE_MARKS.append((int(nc.next_id()), label))
    except Exception:
        pass


def _off(dk):
    """Leading fully-masked columns of a causal diagonal block."""
    return dk * 128 if 1 <= dk <= 3 else 0


def _emit_a(nc, d, it, cst, wts, big, xt_pool, tmp_pool, ps_q, ps_sw, ps_v,
            bcst=None):
    """Project Q^T/K^T (rotary fused) and V for seq columns [it*ST, it*ST+ST).

    For it==0, constant/weight DMAs are sequenced between compute emission
    points so nothing clogs the DMA queues ahead of the first matmuls."""
    s0 = it * ST
    qt_sb, kt_sb, v_sb, _ = big
    wq_sb, wk_sb, wv_sb, _ = wts
    cos_sb, sin_sb, bq_sb, bk_sb, bvb_sb, perm_sb = cst

    xtr_v = d["xtr"].rearrange("(ko p) n -> p ko n", p=128)
    xt = xt_pool.tile([128, KSL, ST], BF16, tag="xt", name="xt")
    if it == 0:
        # first small x/wq chunks up front so the first matmul starts after
        # ~250KB of DMA; HWDGE issue is serial (~625ns per dma_start) so
        # everything else ships in big chunks behind them
        wqv = d["wq"].rearrange("(ko p) i -> p ko i", p=128)
        first = True
        for ks in (slice(0, 1), slice(1, 3), slice(3, 7), slice(7, 11),
                   slice(11, 16)):
            nc.sync.dma_start(xt[:, ks, :], xtr_v[:, ks, s0:s0 + ST])
            nc.sync.dma_start(wq_sb[:, ks, :], wqv[:, ks, :])
            if first:
                # small epilogue constants ride just behind the first chunk
                nc.sync.dma_start(bq_sb[:], d["bq"][:])
                nc.sync.dma_start(bk_sb[:], d["bk"][:])
                nc.sync.dma_start(perm_sb[:], d["perm"][:])
                first = False
        # needed by the first delayed epilogue (~10us in), ahead of wk/wv
        nc.sync.dma_start(cos_sb[:], d["cos_t"][:])
        nc.sync.dma_start(sin_sb[:], d["sin_t"][:])
    else:
        for ka in range(2):
            ks = slice(8 * ka, 8 * (ka + 1))
            nc.sync.dma_start(xt[:, ks, :], xtr_v[:, ks, s0:s0 + ST])

    # delayed epilogues: rotary runs two m-groups behind the projection
    # matmuls so PE never waits on the DVE chain feeding the perm matmul
    epi_q = []

    def _epilogue(pq, b_sb, dst, m):
        qtmp = tmp_pool.tile([128, ST], BF16, tag="qtmp")
        nc.vector.tensor_scalar_add(qtmp[:], pq[:], b_sb[:, m:m + 1])
        psw = ps_sw.tile([128, ST], F32, tag="psw")
        nc.tensor.matmul(psw[:], perm_sb[:], qtmp[:], start=True, stop=True)
        t1 = tmp_pool.tile([128, ST], BF16, tag="t1")
        nc.vector.tensor_mul(t1[:], qtmp[:], cos_sb[:, s0:s0 + ST])
        t2 = tmp_pool.tile([128, ST], BF16, tag="t2")
        nc.vector.tensor_mul(t2[:], psw[:], sin_sb[:, s0:s0 + ST])
        nc.vector.tensor_add(dst[:, m, s0:s0 + ST], t1[:], t2[:])

    for wi, (w_sb, b_sb, dst) in enumerate(((wq_sb, bq_sb, qt_sb),
                                            (wk_sb, bk_sb, kt_sb))):
        if it == 0 and wi == 1:
            wkv = d["wk"].rearrange("(ko p) i -> p ko i", p=128)
            wvv = d["wv"].rearrange("(ko p) i -> p ko i", p=128)
            w_eng = nc.scalar if CONFIG["w_queue"] == "act" else nc.sync
            for ka in range(2):
                ks = slice(8 * ka, 8 * (ka + 1))
                w_eng.dma_start(wk_sb[:, ks, :], wkv[:, ks, :])
                w_eng.dma_start(wv_sb[:, ks, :], wvv[:, ks, :])
            if bcst is not None:
                nc.sync.dma_start(bcst[0][:], d["mask"][:])
                nc.sync.dma_start(bcst[1][:], d["ones"][:])
        for m in range(HPC):
            pq = ps_q.tile([128, ST], F32, tag="pq")
            for k in range(KSL):
                nc.tensor.matmul(
                    pq[:], w_sb[:, k, m * 128:(m + 1) * 128], xt[:, k, :],
                    start=(k == 0), stop=(k == KSL - 1))
            epi_q.append((pq, b_sb, dst, m))
            while len(epi_q) > 1:
                _epilogue(*epi_q.pop(0))

    if it == 0:
        nc.sync.dma_start(bvb_sb[:], d["bvb"][:])
    for sb in range(ST // 128):
        pv = ps_v.tile([128, INNER_C], F32, tag="pv")
        for k in range(KSL):
            nc.tensor.matmul(
                pv[:], xt[:, k, sb * 128:(sb + 1) * 128], wv_sb[:, k, :],
                start=(k == 0), stop=(k == KSL - 1))
        if epi_q:
            _epilogue(*epi_q.pop(0))
        nc.vector.tensor_add(v_sb[:, it * (ST // 128) + sb, :], pv[:], bvb_sb[:])
    for item in epi_q:
        _epilogue(*item)
    epi_q.clear()


def _emit_s_pair(nc, it, jb0, h, cst, big, ps_pool, pt_pool):
    """S matmuls + exp (+ causal mask) for key-block pair (jb0, jb0+1) of
    head h in query block it. Returns a flush-queue entry with po/pr unset."""
    qt_sb, kt_sb = big[0], big[1]
    mask_sb = cst[0]
    i0 = it * ST
    dk0 = jb0 - 4 * it
    o0, o1 = _off(dk0), _off(dk0 + 1)
    ps_pair = ps_pool.tile([128, 2, ST], F32, tag="ps")
    # block 1 also computes from o0 (its [o0:o1) is causally masked to
    # zero below) so the paired exp never reads unwritten PSUM
    nc.tensor.matmul(
        ps_pair[:, 0, o0:], kt_sb[:, h, jb0 * 128:(jb0 + 1) * 128],
        qt_sb[:, h, i0 + o0:i0 + ST], start=True, stop=True)
    nc.tensor.matmul(
        ps_pair[:, 1, o0:], kt_sb[:, h, (jb0 + 1) * 128:(jb0 + 2) * 128],
        qt_sb[:, h, i0 + o0:i0 + ST], start=True, stop=True)
    p_t = pt_pool.tile([128, 2, ST], BF16, tag="p")
    nc.scalar.activation(p_t[:, :, o0:], ps_pair[:, :, o0:], EXP)
    if dk0 >= 0:
        nc.vector.tensor_mul(
            p_t[:, :, o0:], p_t[:, :, o0:], mask_sb[:, dk0:dk0 + 2, o0:])
    return (jb0, o0, o1, p_t, None, None, 0)


def _emit_b(nc, it, cst, big, pt_pool, rec_pool, ps_s, ps_o, ps_r):
    """Causal attention for query block it: S^T = K^T.T @ Q^T per 128-key
    block (paired into 2-bank PSUM tiles for one exp each), exp on ACT,
    diagonal masks on DVE, O^T/rowsum accumulation on PE, normalize.

    The O/rowsum flush queue is software-pipelined across the head loop so
    PE never drains waiting for a fresh head's first exp."""
    qt_sb, kt_sb, v_sb, ot_sb = big
    mask_sb, ones_sb = cst
    i0 = it * ST
    njb = 4 * it + 4

    def _flush(jb0, o0, o1, p_t, po_t, pr_t, h):
        # ones,ones then v,v: the `ones` stationary stays loaded in the PE
        # for the second block; each PSUM tile still accumulates in jb order
        last = jb0 + 1 == njb - 1
        nc.tensor.matmul(
            po_t[:, o0:], v_sb[:, jb0, h * 128:(h + 1) * 128], p_t[:, 0, o0:],
            start=(jb0 == 0), stop=False)
        nc.tensor.matmul(
            po_t[:, o1:], v_sb[:, jb0 + 1, h * 128:(h + 1) * 128],
            p_t[:, 1, o1:],
            start=False, stop=last)
        nc.tensor.matmul(
            pr_t[:, o0:], ones_sb[:], p_t[:, 0, o0:],
            start=(jb0 == 0), stop=False)
        nc.tensor.matmul(
            pr_t[:, o1:], ones_sb[:], p_t[:, 1, o1:],
            start=False, stop=last)
        if last:
            # approx reciprocal (~18 bits) is plenty for softmax denominators
            # and ~5x cheaper on DVE than InstReciprocal
            rec = rec_pool.tile([128, ST], F32, tag="rec")
            nc.vector.reciprocal_approx_fast(rec[:], pr_t[:])
            nc.vector.tensor_mul(ot_sb[:, h, i0:i0 + ST], po_t[:], rec[:])

    pending = []
    for h in range(HPC):
        po_t = ps_o.tile([128, ST], F32, tag="po")
        pr_t = ps_r.tile([128, ST], F32, tag="pr")
        for pri in range(njb // 2):
            jb0 = 2 * pri
            entry = _emit_s_pair(nc, it, jb0, h, cst, big, ps_s, pt_pool)
            pending.append(entry[:4] + (po_t, pr_t, h))
            while len(pending) > 2:
                _flush(*pending.pop(0))
    for item in pending:
        _flush(*item)


def _emit_c(nc, d, it, big, wts, osb_pool, ps_out):
    """Output projection for the 4 seq row-chunks of query block it."""
    ot_sb = big[3]
    wo_sb = wts[3]
    out_d = d["out"]
    last = it == NIT - 1
    for so in range(it * 4, it * 4 + 4):
        if CONFIG["out_queue"] == "both":
            dma_eng = nc.scalar if so % 2 else nc.sync
        else:
            dma_eng = nc.scalar if CONFIG["out_queue"] == "act" else nc.sync
        osb = osb_pool.tile([128, D], BF16, tag="osb")
        split = last and so >= it * 4 + 2
        if CONFIG.get("c_pair"):
            # two matmul chains fill halves of one 2-bank tile; one paired
            # copy drains both (halves the per-instruction copy overhead)
            for np_ in range(D // 1024):
                pout = ps_out.tile([128, 2, 512], F32, tag="poutp")
                for j in range(2):
                    for hh in range(HPC):
                        nc.tensor.matmul(
                            pout[:, j, :],
                            ot_sb[:, hh, so * 128:(so + 1) * 128],
                            wo_sb[:, hh, (2 * np_ + j) * 512:
                                  (2 * np_ + j + 1) * 512],
                            start=(hh == 0), stop=(hh == HPC - 1))
                dst = osb[:, np_ * 1024:(np_ + 1) * 1024]
                if CONFIG["c_copy"] == "alt" and np_ % 2 == 1:
                    nc.scalar.activation(dst, pout[:],
                                         mybir.ActivationFunctionType.Copy)
                else:
                    nc.vector.tensor_copy(dst, pout[:])
                if split or CONFIG.get("out_grain") == 1024:
                    dma_eng.dma_start(
                        out_d[so * 128:(so + 1) * 128,
                              np_ * 1024:(np_ + 1) * 1024], dst)
        else:
            for nt in range(D // 512):
                pout = ps_out.tile([128, 512], F32, tag="pout")
                for hh in range(HPC):
                    nc.tensor.matmul(
                        pout[:], ot_sb[:, hh, so * 128:(so + 1) * 128],
                        wo_sb[:, hh, nt * 512:(nt + 1) * 512],
                        start=(hh == 0), stop=(hh == HPC - 1))
                # alternate the PSUM drain between DVE and ACT so neither
                # becomes the copy bottleneck behind the matmul chains
                if CONFIG["c_copy"] == "alt" and nt % 2 == 1:
                    nc.scalar.activation(osb[:, nt * 512:(nt + 1) * 512],
                                         pout[:],
                                         mybir.ActivationFunctionType.Copy)
                else:
                    nc.vector.tensor_copy(osb[:, nt * 512:(nt + 1) * 512],
                                          pout[:])
                if split:
                    # pipeline the final chunk's copy->DMA to shorten the tail
                    dma_eng.dma_start(
                        out_d[so * 128:(so + 1) * 128,
                              nt * 512:(nt + 1) * 512],
                        osb[:, nt * 512:(nt + 1) * 512])
        if not split and not (CONFIG.get("c_pair")
                              and CONFIG.get("out_grain") == 1024):
            dma_eng.dma_start(out_d[so * 128:(so + 1) * 128, :], osb[:])


def _build_program(phases=("A", "B", "C"), repeat=1):
    PHASE_MARKS.clear()
    phases = {{"A1": "A", "A2": "A"}.get(p, p) for p in phases}
    nc = bacc.Bacc("TRN2", target_bir_lowering=False, debug=False,
                   num_devices=NCORES)

    d = {}
    d["xtr"] = nc.dram_tensor("xtr", [D, N], BF16, kind="ExternalInput").ap()
    d["wq"] = nc.dram_tensor("wq", [D, INNER_C], BF16, kind="ExternalInput").ap()
    d["wk"] = nc.dram_tensor("wk", [D, INNER_C], BF16, kind="ExternalInput").ap()
    d["wv"] = nc.dram_tensor("wv", [D, INNER_C], BF16, kind="ExternalInput").ap()
    d["wo"] = nc.dram_tensor("wo", [INNER_C, D], BF16, kind="ExternalInput").ap()
    d["bq"] = nc.dram_tensor("bq", [128, HPC], F32, kind="ExternalInput").ap()
    d["bk"] = nc.dram_tensor("bk", [128, HPC], F32, kind="ExternalInput").ap()
    d["bvb"] = nc.dram_tensor("bvb", [128, INNER_C], F32, kind="ExternalInput").ap()
    d["cos_t"] = nc.dram_tensor("cos_t", [128, N], BF16, kind="ExternalInput").ap()
    d["sin_t"] = nc.dram_tensor("sin_t", [128, N], F32, kind="ExternalInput").ap()
    d["mask"] = nc.dram_tensor("mask", [128, 4, 512], BF16, kind="ExternalInput").ap()
    d["ones"] = nc.dram_tensor("ones", [128, 128], BF16, kind="ExternalInput").ap()
    d["perm"] = nc.dram_tensor("perm", [128, 128], BF16, kind="ExternalInput").ap()
    d["out"] = nc.dram_tensor("out", [N, D], BF16, kind="ExternalOutput").ap()

    with tile.TileContext(nc) as tc:
        for rep in range(repeat):
            sx = f"_{rep}" if rep else ""
            with (
                tc.tile_pool(name="cst" + sx, bufs=1) as cst_pool,
                tc.tile_pool(name="wts" + sx, bufs=1) as wts_pool,
                tc.tile_pool(name="big" + sx, bufs=1) as big_pool,
                tc.tile_pool(name="xt" + sx, bufs=2) as xt_pool,
                tc.tile_pool(name="tmp" + sx, bufs=2) as tmp_pool,
                tc.tile_pool(name="pt" + sx, bufs=5) as pt_pool,
                tc.tile_pool(name="rec" + sx, bufs=2) as rec_pool,
                tc.tile_pool(name="osb" + sx,
                             bufs=CONFIG["osb_bufs"]) as osb_pool,
            ):
                cos_sb = cst_pool.tile([128, N], BF16)
                sin_sb = cst_pool.tile([128, N], F32)
                bq_sb = cst_pool.tile([128, HPC], F32)
                bk_sb = cst_pool.tile([128, HPC], F32)
                bvb_sb = cst_pool.tile([128, INNER_C], F32)
                perm_sb = cst_pool.tile([128, 128], BF16)
                mask_sb = cst_pool.tile([128, 4, 512], BF16)
                ones_sb = cst_pool.tile([128, 128], BF16)
                if "A" not in phases:
                    # _emit_a normally sequences these between its compute
                    for t, key in ((cos_sb, "cos_t"), (sin_sb, "sin_t"),
                                   (bq_sb, "bq"), (bk_sb, "bk"),
                                   (bvb_sb, "bvb"), (perm_sb, "perm")):
                        nc.sync.dma_start(t[:], d[key][:])

                wq_sb = wts_pool.tile([128, KSL, INNER_C], BF16)
                wk_sb = wts_pool.tile([128, KSL, INNER_C], BF16)
                wv_sb = wts_pool.tile([128, KSL, INNER_C], BF16)
                wo_sb = wts_pool.tile([128, HPC, D], BF16)
                wts = (wq_sb, wk_sb, wv_sb, wo_sb)

                qt_sb = big_pool.tile([128, HPC, N], BF16)
                kt_sb = big_pool.tile([128, HPC, N], BF16)
                v_sb = big_pool.tile([128, NJB, INNER_C], BF16)
                ot_sb = big_pool.tile([128, HPC, N], BF16)
                big = (qt_sb, kt_sb, v_sb, ot_sb)

                if "A" not in phases:
                    nc.gpsimd.memset(qt_sb[:], 0.0)
                    nc.gpsimd.memset(kt_sb[:], 0.0)
                    nc.gpsimd.memset(v_sb[:], 0.0)
                if "B" not in phases:
                    nc.gpsimd.memset(ot_sb[:], 0.0)

                acst = (cos_sb, sin_sb, bq_sb, bk_sb, bvb_sb, perm_sb)
                bcst = (mask_sb, ones_sb)

                def emit_c(it):
                    _mark(nc, f"C{it}{sx}")
                    with tc.tile_pool(name=f"ps_out{it}" + sx,
                                      bufs=CONFIG["ps_out_bufs"],
                                      space="PSUM") as ps_out:
                        _emit_c(nc, d, it, big, wts, osb_pool, ps_out)

                for it in range(NIT):
                    if "A" in phases:
                        _mark(nc, f"A{it}{sx}")
                        with (
                            tc.tile_pool(name=f"ps_q{it}" + sx, bufs=3,
                                         space="PSUM") as ps_q,
                            tc.tile_pool(name=f"ps_sw{it}" + sx, bufs=1,
                                         space="PSUM") as ps_sw,
                            tc.tile_pool(name=f"ps_v{it}" + sx, bufs=2,
                                         space="PSUM") as ps_v,
                        ):
                            _emit_a(nc, d, it, acst, wts, big, xt_pool,
                                    tmp_pool, ps_q, ps_sw, ps_v,
                                    bcst=bcst)
                    if it == 0:
                        if "A" not in phases:
                            nc.sync.dma_start(mask_sb[:], d["mask"][:])
                            nc.sync.dma_start(ones_sb[:], d["ones"][:])
                        if "C" in phases:
                            w_eng2 = (nc.scalar if CONFIG["w_queue"] == "act"
                                      else nc.sync)
                            w_eng2.dma_start(
                                wo_sb[:],
                                d["wo"].rearrange("(h p) n -> p h n", p=128)[:])
                    if "B" in phases:
                        _mark(nc, f"B{it}{sx}")
                        with (
                            tc.tile_pool(name=f"ps_s{it}" + sx, bufs=2,
                                         space="PSUM") as ps_s,
                            tc.tile_pool(name=f"ps_o{it}" + sx, bufs=2,
                                         space="PSUM") as ps_o,
                            tc.tile_pool(name=f"ps_r{it}" + sx, bufs=2,
                                         space="PSUM") as ps_r,
                        ):
                            _emit_b(nc, it, bcst, big, pt_pool, rec_pool,
                                    ps_s, ps_o, ps_r)
                    if "C" in phases:
                        emit_c(it)

    nc.compile()
    return nc


def _host_consts():
    scale = DH ** -0.5
    inv_freq = 1.0 / (10000.0 ** (np.arange(0, DH, 2, dtype=np.float32) / DH))
    seq = np.arange(N, dtype=np.float32)
    freqs = np.einsum('i,j->ij', seq, inv_freq)          # [N, 64]
    pos = np.concatenate((freqs, freqs), axis=-1)        # [N, 128]
    cos_t = np.cos(pos).T.astype(NPBF).copy()            # [128, N] bf16
    sin_t = np.sin(pos).T.astype(np.float32)             # [128, N] f32
    sin_t[:64] *= -1.0                                   # rotate_half sign fold

    perm = np.zeros((128, 128), dtype=np.float32)
    perm[(np.arange(128) + 64) % 128, np.arange(128)] = 1.0

    mask = np.zeros((128, 4, 512), dtype=np.float32)
    jj = np.arange(128)[:, None]
    ii = np.arange(512)[None, :]
    for dk in range(4):
        mask[:, dk, :] = (jj + dk * 128 <= ii)

    ones = np.ones((128, 128), dtype=np.float32)
    return scale, cos_t, sin_t, perm.astype(NPBF), mask.astype(NPBF), \
        ones.astype(NPBF)


def make_in_maps(x, Wq, bq, Wk, bk, Wv, bv, Wo, bo):
    x = np.asarray(x, dtype=np.float32)
    Wq = np.asarray(Wq, dtype=np.float32)
    Wk = np.asarray(Wk, dtype=np.float32)
    Wv = np.asarray(Wv, dtype=np.float32)
    Wo = np.asarray(Wo, dtype=np.float32)
    bq = np.asarray(bq, dtype=np.float32)
    bk = np.asarray(bk, dtype=np.float32)
    bv = np.asarray(bv, dtype=np.float32)

    scale, cos_t, sin_t, perm, mask, ones = _host_consts()

    in_maps = []
    for c in range(NCORES):
        b, g = c // GROUPS, c % GROUPS
        sl = slice(g * INNER_C, (g + 1) * INNER_C)
        in_maps.append({
            "xtr": np.ascontiguousarray(x[b].T).astype(NPBF),
            "wq": np.ascontiguousarray(Wq[:, sl] * scale).astype(NPBF),
            "wk": np.ascontiguousarray(Wk[:, sl]).astype(NPBF),
            "wv": np.ascontiguousarray(Wv[:, sl]).astype(NPBF),
            "wo": np.ascontiguousarray(Wo[sl, :]).astype(NPBF),
            "bq": np.ascontiguousarray((bq[sl] * scale).reshape(HPC, 128).T),
            "bk": np.ascontiguousarray(bk[sl].reshape(HPC, 128).T),
            "bvb": np.ascontiguousarray(np.tile(bv[sl], (128, 1))),
            "cos_t": cos_t,
            "sin_t": sin_t,
            "mask": mask,
            "ones": ones,
            "perm": perm,
        })
    return in_maps


def kernel(x, Wq, bq, Wk, bk, Wv, bv, Wo, bo):
    global LAST_RESULTS
    if "nc" not in _CACHE:
        _CACHE["nc"] = _build_program()
    nc = _CACHE["nc"]

    bo = np.asarray(bo, dtype=np.float32)
    in_maps = make_in_maps(x, Wq, bq, Wk, bk, Wv, bv, Wo, bo)

    LAST_RESULTS = run_bass_kernel_spmd(nc, in_maps, core_ids=list(range(NCORES)))
    results = LAST_RESULTS.results

    out = np.zeros((B, N, D), dtype=np.float32)
    for c in range(NCORES):
        out[c // GROUPS] += results[c]["out"].astype(np.float32)
    out += bo
    return out



# revision 21
# speedup vs baseline: 1.0007x; 1.0007x over previous
"""Trainium2 Bass kernel for nn_Attention_85856396247857.

16-head causal attention with rotary embeddings, fp32 in/out, x:[2,2048,2048].

Sharding (8 cores): core c = (b, g) with b = c // 4 (batch), g = c % 4
(head group). Each core handles batch b and heads 4g..4g+3 (tensor
parallel: Wq/Wk/Wv column-sliced by head, Wo row-sliced; the row-parallel
output partials are summed on the host).

v2: all operands bf16 (PSUM accumulation stays f32) — bf16 moving operands
stream ~2 cols/cycle on TRN2 (measured ~130ns per 512-col LDW+MM pair), so
the projection phases run at ~2x the f32 rate. Single fused pass over x
computes Q^T/K^T (rotary fused, rotate_half via a 128x128 permutation
matmul) and V; flash-style attention in S^T layout with paired-block exp
instructions and causal column-offset trimming on all diagonal blocks;
output projection interleaved per 512-query tile.

v3 (measured-bottleneck round): ps_o/ps_r double-buffered so consecutive
heads' flush chains never wait on the previous head's normalize;
reciprocal_approx_fast (~5x cheaper on DVE, 18-bit accurate — plenty for
softmax denominators); C-phase drains two 512-col PSUM chunks per copy
instruction, alternating DVE/ACT; output DMAs ride the Activation HWDGE
queue so they never queue behind x/weight input traffic on the SP queue.
HW phase split (R=17 amplification): A ~106us (PE-bound at the measured
~130ns/512-col-MM stream rate), B ~69us (ACT exp-bound), C ~70us.
Run-to-run variance across processes is +-40us (HBM buffer placement
lottery in PJRT allocation) — config deltas below ~20us are noise.

Numerics: bf16 rounding gives rel err ~3.5e-3 vs the f32 reference
(tolerance 2e-2); exp needs no max-subtraction since |S| < 6.

`phases` / `repeat` exist for benchmarking (differential phase timing and
in-NEFF amplification); production uses the defaults.
"""

import os
import sys

import numpy as np

for _p in ("/opt/trn_rl_repo",):
    if _p not in sys.path and os.path.isdir(_p):
        sys.path.insert(0, _p)

import ml_dtypes  # noqa: E402

import concourse.bass as bass  # noqa: E402
import concourse.mybir as mybir  # noqa: E402
import concourse.tile as tile  # noqa: E402
from concourse import bacc  # noqa: E402
from concourse.bass_utils import run_bass_kernel_spmd  # noqa: E402

F32 = mybir.dt.float32
BF16 = mybir.dt.bfloat16
NPBF = ml_dtypes.bfloat16
EXP = mybir.ActivationFunctionType.Exp

# Problem shape (hardcoded per contract)
B, N, D = 2, 2048, 2048
H, DH = 16, 128
NCORES = 8
GROUPS = 4              # head groups (tensor parallel)
HPC = H // GROUPS       # heads per core = 4
INNER_C = HPC * DH      # per-core inner dim = 512

KSL = D // 128          # 16 contraction slices
ST = 512                # seq columns per tile / query-block granularity
NIT = N // ST           # 4
NJB = N // 128          # 16 key blocks

_CACHE = {}
LAST_RESULTS = None
PHASE_MARKS = []  # (instruction-id watermark, label) per build, for simbench

# Tunables for A/B benchmarking of scheduling variants (bench_variants.py).
CONFIG = {
    "c_copy": "alt",      # "alt" (DVE/ACT alternate) | "dve" (all DVE)
    "out_queue": "act",   # "sp" | "act" | "both" — queue(s) for C's out DMAs
    "w_queue": "sp",      # "sp" | "act" — queue for wk/wv/wo weight loads
    "osb_bufs": 4,        # out-staging buffer depth (4 x 1024-col chunks)
    "ps_out_bufs": 4,     # C-phase PSUM buffer depth
    "c_pair": 1,          # drain two 512-col PSUM chunks per copy inst
    "osb_chunk": 1,       # per-1024-col staging buffers + per-chunk DMA
}


def _mark(nc, label):
    try:
        PHASE_MARKS.append((int(nc.next_id()), label))
    except Exception:
        pass


def _off(dk):
    """Leading fully-masked columns of a causal diagonal block."""
    return dk * 128 if 1 <= dk <= 3 else 0


def _emit_a(nc, d, it, cst, wts, big, xt_pool, tmp_pool, ps_q, ps_sw, ps_v,
            bcst=None):
    """Project Q^T/K^T (rotary fused) and V for seq columns [it*ST, it*ST+ST).

    For it==0, constant/weight DMAs are sequenced between compute emission
    points so nothing clogs the DMA queues ahead of the first matmuls."""
    s0 = it * ST
    qt_sb, kt_sb, v_sb, _ = big
    wq_sb, wk_sb, wv_sb, _ = wts
    cos_sb, sin_sb, bq_sb, bk_sb, bvb_sb, perm_sb = cst

    xtr_v = d["xtr"].rearrange("(ko p) n -> p ko n", p=128)
    xt = xt_pool.tile([128, KSL, ST], BF16, tag="xt", name="xt")
    if it == 0:
        # first small x/wq chunks up front so the first matmul starts after
        # ~250KB of DMA; HWDGE issue is serial (~625ns per dma_start) so
        # everything else ships in big chunks behind them
        wqv = d["wq"].rearrange("(ko p) i -> p ko i", p=128)
        first = True
        for ks in (slice(0, 1), slice(1, 3), slice(3, 7), slice(7, 11),
                   slice(11, 16)):
            nc.sync.dma_start(xt[:, ks, :], xtr_v[:, ks, s0:s0 + ST])
            nc.sync.dma_start(wq_sb[:, ks, :], wqv[:, ks, :])
            if first:
                # small epilogue constants ride just behind the first chunk
                nc.sync.dma_start(bq_sb[:], d["bq"][:])
                nc.sync.dma_start(bk_sb[:], d["bk"][:])
                nc.sync.dma_start(perm_sb[:], d["perm"][:])
                first = False
        # needed by the first delayed epilogue (~10us in), ahead of wk/wv
        nc.sync.dma_start(cos_sb[:], d["cos_t"][:])
        nc.sync.dma_start(sin_sb[:], d["sin_t"][:])
    else:
        for ka in range(2):
            ks = slice(8 * ka, 8 * (ka + 1))
            nc.sync.dma_start(xt[:, ks, :], xtr_v[:, ks, s0:s0 + ST])

    # delayed epilogues: rotary runs two m-groups behind the projection
    # matmuls so PE never waits on the DVE chain feeding the perm matmul
    epi_q = []

    def _epilogue(pq, b_sb, dst, m):
        qtmp = tmp_pool.tile([128, ST], BF16, tag="qtmp")
        nc.vector.tensor_scalar_add(qtmp[:], pq[:], b_sb[:, m:m + 1])
        psw = ps_sw.tile([128, ST], F32, tag="psw")
        nc.tensor.matmul(psw[:], perm_sb[:], qtmp[:], start=True, stop=True)
        t1 = tmp_pool.tile([128, ST], BF16, tag="t1")
        nc.vector.tensor_mul(t1[:], qtmp[:], cos_sb[:, s0:s0 + ST])
        t2 = tmp_pool.tile([128, ST], BF16, tag="t2")
        nc.vector.tensor_mul(t2[:], psw[:], sin_sb[:, s0:s0 + ST])
        nc.vector.tensor_add(dst[:, m, s0:s0 + ST], t1[:], t2[:])

    for wi, (w_sb, b_sb, dst) in enumerate(((wq_sb, bq_sb, qt_sb),
                                            (wk_sb, bk_sb, kt_sb))):
        if it == 0 and wi == 1:
            wkv = d["wk"].rearrange("(ko p) i -> p ko i", p=128)
            wvv = d["wv"].rearrange("(ko p) i -> p ko i", p=128)
            w_eng = nc.scalar if CONFIG["w_queue"] == "act" else nc.sync
            for ka in range(2):
                ks = slice(8 * ka, 8 * (ka + 1))
                w_eng.dma_start(wk_sb[:, ks, :], wkv[:, ks, :])
                w_eng.dma_start(wv_sb[:, ks, :], wvv[:, ks, :])
            if bcst is not None:
                nc.sync.dma_start(bcst[0][:], d["mask"][:])
                nc.sync.dma_start(bcst[1][:], d["ones"][:])
        for m in range(HPC):
            pq = ps_q.tile([128, ST], F32, tag="pq")
            for k in range(KSL):
                nc.tensor.matmul(
                    pq[:], w_sb[:, k, m * 128:(m + 1) * 128], xt[:, k, :],
                    start=(k == 0), stop=(k == KSL - 1))
            epi_q.append((pq, b_sb, dst, m))
            while len(epi_q) > 1:
                _epilogue(*epi_q.pop(0))

    if it == 0:
        nc.sync.dma_start(bvb_sb[:], d["bvb"][:])
    for sb in range(ST // 128):
        pv = ps_v.tile([128, INNER_C], F32, tag="pv")
        for k in range(KSL):
            nc.tensor.matmul(
                pv[:], xt[:, k, sb * 128:(sb + 1) * 128], wv_sb[:, k, :],
                start=(k == 0), stop=(k == KSL - 1))
        if epi_q:
            _epilogue(*epi_q.pop(0))
        nc.vector.tensor_add(v_sb[:, it * (ST // 128) + sb, :], pv[:], bvb_sb[:])
    for item in epi_q:
        _epilogue(*item)
    epi_q.clear()


def _emit_s_pair(nc, it, jb0, h, cst, big, ps_pool, pt_pool):
    """S matmuls + exp (+ causal mask) for key-block pair (jb0, jb0+1) of
    head h in query block it. Returns a flush-queue entry with po/pr unset."""
    qt_sb, kt_sb = big[0], big[1]
    mask_sb = cst[0]
    i0 = it * ST
    dk0 = jb0 - 4 * it
    o0, o1 = _off(dk0), _off(dk0 + 1)
    ps_pair = ps_pool.tile([128, 2, ST], F32, tag="ps")
    # block 1 also computes from o0 (its [o0:o1) is causally masked to
    # zero below) so the paired exp never reads unwritten PSUM
    nc.tensor.matmul(
        ps_pair[:, 0, o0:], kt_sb[:, h, jb0 * 128:(jb0 + 1) * 128],
        qt_sb[:, h, i0 + o0:i0 + ST], start=True, stop=True)
    nc.tensor.matmul(
        ps_pair[:, 1, o0:], kt_sb[:, h, (jb0 + 1) * 128:(jb0 + 2) * 128],
        qt_sb[:, h, i0 + o0:i0 + ST], start=True, stop=True)
    p_t = pt_pool.tile([128, 2, ST], BF16, tag="p")
    nc.scalar.activation(p_t[:, :, o0:], ps_pair[:, :, o0:], EXP)
    if dk0 >= 0:
        nc.vector.tensor_mul(
            p_t[:, :, o0:], p_t[:, :, o0:], mask_sb[:, dk0:dk0 + 2, o0:])
    return (jb0, o0, o1, p_t, None, None, 0)


def _emit_b(nc, it, cst, big, pt_pool, rec_pool, ps_s, ps_o, ps_r):
    """Causal attention for query block it: S^T = K^T.T @ Q^T per 128-key
    block (paired into 2-bank PSUM tiles for one exp each), exp on ACT,
    diagonal masks on DVE, O^T/rowsum accumulation on PE, normalize.

    The O/rowsum flush queue is software-pipelined across the head loop so
    PE never drains waiting for a fresh head's first exp."""
    qt_sb, kt_sb, v_sb, ot_sb = big
    mask_sb, ones_sb = cst
    i0 = it * ST
    njb = 4 * it + 4

    def _flush(jb0, o0, o1, p_t, po_t, pr_t, h):
        # ones,ones then v,v: the `ones` stationary stays loaded in the PE
        # for the second block; each PSUM tile still accumulates in jb order
        last = jb0 + 1 == njb - 1
        nc.tensor.matmul(
            po_t[:, o0:], v_sb[:, jb0, h * 128:(h + 1) * 128], p_t[:, 0, o0:],
            start=(jb0 == 0), stop=False)
        nc.tensor.matmul(
            po_t[:, o1:], v_sb[:, jb0 + 1, h * 128:(h + 1) * 128],
            p_t[:, 1, o1:],
            start=False, stop=last)
        nc.tensor.matmul(
            pr_t[:, o0:], ones_sb[:], p_t[:, 0, o0:],
            start=(jb0 == 0), stop=False)
        nc.tensor.matmul(
            pr_t[:, o1:], ones_sb[:], p_t[:, 1, o1:],
            start=False, stop=last)
        if last:
            # approx reciprocal (~18 bits) is plenty for softmax denominators
            # and ~5x cheaper on DVE than InstReciprocal
            rec = rec_pool.tile([128, ST], F32, tag="rec")
            nc.vector.reciprocal_approx_fast(rec[:], pr_t[:])
            nc.vector.tensor_mul(ot_sb[:, h, i0:i0 + ST], po_t[:], rec[:])

    pending = []
    for h in range(HPC):
        po_t = ps_o.tile([128, ST], F32, tag="po")
        pr_t = ps_r.tile([128, ST], F32, tag="pr")
        for pri in range(njb // 2):
            jb0 = 2 * pri
            entry = _emit_s_pair(nc, it, jb0, h, cst, big, ps_s, pt_pool)
            pending.append(entry[:4] + (po_t, pr_t, h))
            while len(pending) > 2:
                _flush(*pending.pop(0))
    for item in pending:
        _flush(*item)


def _emit_c(nc, d, it, big, wts, osb_pool, ps_out):
    """Output projection for the 4 seq row-chunks of query block it."""
    ot_sb = big[3]
    wo_sb = wts[3]
    out_d = d["out"]
    last = it == NIT - 1
    for so in range(it * 4, it * 4 + 4):
        if CONFIG["out_queue"] == "both":
            dma_eng = nc.scalar if so % 2 else nc.sync
        else:
            dma_eng = nc.scalar if CONFIG["out_queue"] == "act" else nc.sync
        chunked = CONFIG.get("c_pair") and CONFIG.get("osb_chunk")
        osb = None if chunked else osb_pool.tile([128, D], BF16, tag="osb")
        split = last and so >= it * 4 + 2
        if CONFIG.get("c_pair"):
            # two matmul chains fill halves of one 2-bank tile; one paired
            # copy drains both (halves the per-instruction copy overhead)
            for np_ in range(D // 1024):
                pout = ps_out.tile([128, 2, 512], F32, tag="poutp")
                for j in range(2):
                    for hh in range(HPC):
                        nc.tensor.matmul(
                            pout[:, j, :],
                            ot_sb[:, hh, so * 128:(so + 1) * 128],
                            wo_sb[:, hh, (2 * np_ + j) * 512:
                                  (2 * np_ + j + 1) * 512],
                            start=(hh == 0), stop=(hh == HPC - 1))
                if chunked:
                    # per-1024 staging buffers + per-chunk DMA: same SBUF
                    # footprint as 2 full-row buffers but 2x finer recycling,
                    # so a copy never waits on a whole row-chunk's DMA drain
                    osbc = osb_pool.tile([128, 1024], BF16, tag="osbc",
                                         name="osbc")
                    dst = osbc[:]
                else:
                    dst = osb[:, np_ * 1024:(np_ + 1) * 1024]
                if CONFIG["c_copy"] == "alt" and np_ % 2 == 1:
                    nc.scalar.activation(dst, pout[:],
                                         mybir.ActivationFunctionType.Copy)
                else:
                    nc.vector.tensor_copy(dst, pout[:])
                if chunked or split or CONFIG.get("out_grain") == 1024:
                    dma_eng.dma_start(
                        out_d[so * 128:(so + 1) * 128,
                              np_ * 1024:(np_ + 1) * 1024], dst)
        else:
            for nt in range(D // 512):
                pout = ps_out.tile([128, 512], F32, tag="pout")
                for hh in range(HPC):
                    nc.tensor.matmul(
                        pout[:], ot_sb[:, hh, so * 128:(so + 1) * 128],
                        wo_sb[:, hh, nt * 512:(nt + 1) * 512],
                        start=(hh == 0), stop=(hh == HPC - 1))
                # alternate the PSUM drain between DVE and ACT so neither
                # becomes the copy bottleneck behind the matmul chains
                if CONFIG["c_copy"] == "alt" and nt % 2 == 1:
                    nc.scalar.activation(osb[:, nt * 512:(nt + 1) * 512],
                                         pout[:],
                                         mybir.ActivationFunctionType.Copy)
                else:
                    nc.vector.tensor_copy(osb[:, nt * 512:(nt + 1) * 512],
                                          pout[:])
                if split:
                    # pipeline the final chunk's copy->DMA to shorten the tail
                    dma_eng.dma_start(
                        out_d[so * 128:(so + 1) * 128,
                              nt * 512:(nt + 1) * 512],
                        osb[:, nt * 512:(nt + 1) * 512])
        if osb is not None and not split and CONFIG.get("out_grain") != 1024:
            dma_eng.dma_start(out_d[so * 128:(so + 1) * 128, :], osb[:])


def _build_program(phases=("A", "B", "C"), repeat=1):
    PHASE_MARKS.clear()
    phases = {{"A1": "A", "A2": "A"}.get(p, p) for p in phases}
    nc = bacc.Bacc("TRN2", target_bir_lowering=False, debug=False,
                   num_devices=NCORES)

    d = {}
    d["xtr"] = nc.dram_tensor("xtr", [D, N], BF16, kind="ExternalInput").ap()
    d["wq"] = nc.dram_tensor("wq", [D, INNER_C], BF16, kind="ExternalInput").ap()
    d["wk"] = nc.dram_tensor("wk", [D, INNER_C], BF16, kind="ExternalInput").ap()
    d["wv"] = nc.dram_tensor("wv", [D, INNER_C], BF16, kind="ExternalInput").ap()
    d["wo"] = nc.dram_tensor("wo", [INNER_C, D], BF16, kind="ExternalInput").ap()
    d["bq"] = nc.dram_tensor("bq", [128, HPC], F32, kind="ExternalInput").ap()
    d["bk"] = nc.dram_tensor("bk", [128, HPC], F32, kind="ExternalInput").ap()
    d["bvb"] = nc.dram_tensor("bvb", [128, INNER_C], F32, kind="ExternalInput").ap()
    d["cos_t"] = nc.dram_tensor("cos_t", [128, N], BF16, kind="ExternalInput").ap()
    d["sin_t"] = nc.dram_tensor("sin_t", [128, N], F32, kind="ExternalInput").ap()
    d["mask"] = nc.dram_tensor("mask", [128, 4, 512], BF16, kind="ExternalInput").ap()
    d["ones"] = nc.dram_tensor("ones", [128, 128], BF16, kind="ExternalInput").ap()
    d["perm"] = nc.dram_tensor("perm", [128, 128], BF16, kind="ExternalInput").ap()
    d["out"] = nc.dram_tensor("out", [N, D], BF16, kind="ExternalOutput").ap()

    with tile.TileContext(nc) as tc:
        for rep in range(repeat):
            sx = f"_{rep}" if rep else ""
            with (
                tc.tile_pool(name="cst" + sx, bufs=1) as cst_pool,
                tc.tile_pool(name="wts" + sx, bufs=1) as wts_pool,
                tc.tile_pool(name="big" + sx, bufs=1) as big_pool,
                tc.tile_pool(name="xt" + sx, bufs=2) as xt_pool,
                tc.tile_pool(name="tmp" + sx, bufs=2) as tmp_pool,
                tc.tile_pool(name="pt" + sx, bufs=5) as pt_pool,
                tc.tile_pool(name="rec" + sx, bufs=2) as rec_pool,
                tc.tile_pool(name="osb" + sx,
                             bufs=CONFIG["osb_bufs"]) as osb_pool,
            ):
                cos_sb = cst_pool.tile([128, N], BF16)
                sin_sb = cst_pool.tile([128, N], F32)
                bq_sb = cst_pool.tile([128, HPC], F32)
                bk_sb = cst_pool.tile([128, HPC], F32)
                bvb_sb = cst_pool.tile([128, INNER_C], F32)
                perm_sb = cst_pool.tile([128, 128], BF16)
                mask_sb = cst_pool.tile([128, 4, 512], BF16)
                ones_sb = cst_pool.tile([128, 128], BF16)
                if "A" not in phases:
                    # _emit_a normally sequences these between its compute
                    for t, key in ((cos_sb, "cos_t"), (sin_sb, "sin_t"),
                                   (bq_sb, "bq"), (bk_sb, "bk"),
                                   (bvb_sb, "bvb"), (perm_sb, "perm")):
                        nc.sync.dma_start(t[:], d[key][:])

                wq_sb = wts_pool.tile([128, KSL, INNER_C], BF16)
                wk_sb = wts_pool.tile([128, KSL, INNER_C], BF16)
                wv_sb = wts_pool.tile([128, KSL, INNER_C], BF16)
                wo_sb = wts_pool.tile([128, HPC, D], BF16)
                wts = (wq_sb, wk_sb, wv_sb, wo_sb)

                qt_sb = big_pool.tile([128, HPC, N], BF16)
                kt_sb = big_pool.tile([128, HPC, N], BF16)
                v_sb = big_pool.tile([128, NJB, INNER_C], BF16)
                ot_sb = big_pool.tile([128, HPC, N], BF16)
                big = (qt_sb, kt_sb, v_sb, ot_sb)

                if "A" not in phases:
                    nc.gpsimd.memset(qt_sb[:], 0.0)
                    nc.gpsimd.memset(kt_sb[:], 0.0)
                    nc.gpsimd.memset(v_sb[:], 0.0)
                if "B" not in phases:
                    nc.gpsimd.memset(ot_sb[:], 0.0)

                acst = (cos_sb, sin_sb, bq_sb, bk_sb, bvb_sb, perm_sb)
                bcst = (mask_sb, ones_sb)

                def emit_c(it):
                    _mark(nc, f"C{it}{sx}")
                    with tc.tile_pool(name=f"ps_out{it}" + sx,
                                      bufs=CONFIG["ps_out_bufs"],
                                      space="PSUM") as ps_out:
                        _emit_c(nc, d, it, big, wts, osb_pool, ps_out)

                for it in range(NIT):
                    if "A" in phases:
                        _mark(nc, f"A{it}{sx}")
                        with (
                            tc.tile_pool(name=f"ps_q{it}" + sx, bufs=3,
                                         space="PSUM") as ps_q,
                            tc.tile_pool(name=f"ps_sw{it}" + sx, bufs=1,
                                         space="PSUM") as ps_sw,
                            tc.tile_pool(name=f"ps_v{it}" + sx, bufs=2,
                                         space="PSUM") as ps_v,
                        ):
                            _emit_a(nc, d, it, acst, wts, big, xt_pool,
                                    tmp_pool, ps_q, ps_sw, ps_v,
                                    bcst=bcst)
                    if it == 0:
                        if "A" not in phases:
                            nc.sync.dma_start(mask_sb[:], d["mask"][:])
                            nc.sync.dma_start(ones_sb[:], d["ones"][:])
                        if "C" in phases:
                            w_eng2 = (nc.scalar if CONFIG["w_queue"] == "act"
                                      else nc.sync)
                            w_eng2.dma_start(
                                wo_sb[:],
                                d["wo"].rearrange("(h p) n -> p h n", p=128)[:])
                    if "B" in phases:
                        _mark(nc, f"B{it}{sx}")
                        with (
                            tc.tile_pool(name=f"ps_s{it}" + sx, bufs=2,
                                         space="PSUM") as ps_s,
                            tc.tile_pool(name=f"ps_o{it}" + sx, bufs=2,
                                         space="PSUM") as ps_o,
                            tc.tile_pool(name=f"ps_r{it}" + sx, bufs=2,
                                         space="PSUM") as ps_r,
                        ):
                            _emit_b(nc, it, bcst, big, pt_pool, rec_pool,
                                    ps_s, ps_o, ps_r)
                    if "C" in phases:
                        emit_c(it)

    nc.compile()
    return nc


def _host_consts():
    scale = DH ** -0.5
    inv_freq = 1.0 / (10000.0 ** (np.arange(0, DH, 2, dtype=np.float32) / DH))
    seq = np.arange(N, dtype=np.float32)
    freqs = np.einsum('i,j->ij', seq, inv_freq)          # [N, 64]
    pos = np.concatenate((freqs, freqs), axis=-1)        # [N, 128]
    cos_t = np.cos(pos).T.astype(NPBF).copy()            # [128, N] bf16
    sin_t = np.sin(pos).T.astype(np.float32)             # [128, N] f32
    sin_t[:64] *= -1.0                                   # rotate_half sign fold

    perm = np.zeros((128, 128), dtype=np.float32)
    perm[(np.arange(128) + 64) % 128, np.arange(128)] = 1.0

    mask = np.zeros((128, 4, 512), dtype=np.float32)
    jj = np.arange(128)[:, None]
    ii = np.arange(512)[None, :]
    for dk in range(4):
        mask[:, dk, :] = (jj + dk * 128 <= ii)

    ones = np.ones((128, 128), dtype=np.float32)
    return scale, cos_t, sin_t, perm.astype(NPBF), mask.astype(NPBF), \
        ones.astype(NPBF)


def make_in_maps(x, Wq, bq, Wk, bk, Wv, bv, Wo, bo):
    x = np.asarray(x, dtype=np.float32)
    Wq = np.asarray(Wq, dtype=np.float32)
    Wk = np.asarray(Wk, dtype=np.float32)
    Wv = np.asarray(Wv, dtype=np.float32)
    Wo = np.asarray(Wo, dtype=np.float32)
    bq = np.asarray(bq, dtype=np.float32)
    bk = np.asarray(bk, dtype=np.float32)
    bv = np.asarray(bv, dtype=np.float32)

    scale, cos_t, sin_t, perm, mask, ones = _host_consts()

    in_maps = []
    for c in range(NCORES):
        b, g = c // GROUPS, c % GROUPS
        sl = slice(g * INNER_C, (g + 1) * INNER_C)
        in_maps.append({
            "xtr": np.ascontiguousarray(x[b].T).astype(NPBF),
            "wq": np.ascontiguousarray(Wq[:, sl] * scale).astype(NPBF),
            "wk": np.ascontiguousarray(Wk[:, sl]).astype(NPBF),
            "wv": np.ascontiguousarray(Wv[:, sl]).astype(NPBF),
            "wo": np.ascontiguousarray(Wo[sl, :]).astype(NPBF),
            "bq": np.ascontiguousarray((bq[sl] * scale).reshape(HPC, 128).T),
            "bk": np.ascontiguousarray(bk[sl].reshape(HPC, 128).T),
            "bvb": np.ascontiguousarray(np.tile(bv[sl], (128, 1))),
            "cos_t": cos_t,
            "sin_t": sin_t,
            "mask": mask,
            "ones": ones,
            "perm": perm,
        })
    return in_maps


def kernel(x, Wq, bq, Wk, bk, Wv, bv, Wo, bo):
    global LAST_RESULTS
    if "nc" not in _CACHE:
        _CACHE["nc"] = _build_program()
    nc = _CACHE["nc"]

    bo = np.asarray(bo, dtype=np.float32)
    in_maps = make_in_maps(x, Wq, bq, Wk, bk, Wv, bv, Wo, bo)

    LAST_RESULTS = run_bass_kernel_spmd(nc, in_maps, core_ids=list(range(NCORES)))
    results = LAST_RESULTS.results

    out = np.zeros((B, N, D), dtype=np.float32)
    for c in range(NCORES):
        out[c // GROUPS] += results[c]["out"].astype(np.float32)
    out += bo
    return out



# revision 22
# speedup vs baseline: 1.0129x; 1.0122x over previous
"""Trainium2 Bass kernel for nn_Attention_85856396247857.

16-head causal attention with rotary embeddings, fp32 in/out, x:[2,2048,2048].

Sharding (8 cores): core c = (b, g) with b = c // 4 (batch), g = c % 4
(head group). Each core handles batch b and heads 4g..4g+3 (tensor
parallel: Wq/Wk/Wv column-sliced by head, Wo row-sliced; the row-parallel
output partials are summed on the host).

v2: all operands bf16 (PSUM accumulation stays f32) — bf16 moving operands
stream ~2 cols/cycle on TRN2 (measured ~130ns per 512-col LDW+MM pair), so
the projection phases run at ~2x the f32 rate. Single fused pass over x
computes Q^T/K^T (rotary fused, rotate_half via a 128x128 permutation
matmul) and V; flash-style attention in S^T layout with paired-block exp
instructions and causal column-offset trimming on all diagonal blocks;
output projection interleaved per 512-query tile.

v3 (measured-bottleneck round): ps_o/ps_r double-buffered so consecutive
heads' flush chains never wait on the previous head's normalize;
reciprocal_approx_fast (~5x cheaper on DVE, 18-bit accurate — plenty for
softmax denominators); C-phase drains two 512-col PSUM chunks per copy
instruction, alternating DVE/ACT; output DMAs ride the Activation HWDGE
queue so they never queue behind x/weight input traffic on the SP queue.
HW phase split (R=17 amplification): A ~106us (PE-bound at the measured
~130ns/512-col-MM stream rate), B ~69us (ACT exp-bound), C ~70us.
Run-to-run variance across processes is +-40us (HBM buffer placement
lottery in PJRT allocation) — config deltas below ~20us are noise.

Numerics: bf16 rounding gives rel err ~3.5e-3 vs the f32 reference
(tolerance 2e-2); exp needs no max-subtraction since |S| < 6.

`phases` / `repeat` exist for benchmarking (differential phase timing and
in-NEFF amplification); production uses the defaults.
"""

import os
import sys

import numpy as np

for _p in ("/opt/trn_rl_repo",):
    if _p not in sys.path and os.path.isdir(_p):
        sys.path.insert(0, _p)

import ml_dtypes  # noqa: E402

import concourse.bass as bass  # noqa: E402
import concourse.mybir as mybir  # noqa: E402
import concourse.tile as tile  # noqa: E402
from concourse import bacc  # noqa: E402
from concourse.bass_utils import run_bass_kernel_spmd  # noqa: E402

F32 = mybir.dt.float32
BF16 = mybir.dt.bfloat16
NPBF = ml_dtypes.bfloat16
EXP = mybir.ActivationFunctionType.Exp

# Problem shape (hardcoded per contract)
B, N, D = 2, 2048, 2048
H, DH = 16, 128
NCORES = 8
GROUPS = 4              # head groups (tensor parallel)
HPC = H // GROUPS       # heads per core = 4
INNER_C = HPC * DH      # per-core inner dim = 512

KSL = D // 128          # 16 contraction slices
ST = 512                # seq columns per tile / query-block granularity
NIT = N // ST           # 4
NJB = N // 128          # 16 key blocks

_CACHE = {}
LAST_RESULTS = None
PHASE_MARKS = []  # (instruction-id watermark, label) per build, for simbench

# Tunables for A/B benchmarking of scheduling variants (bench_variants.py).
CONFIG = {
    "c_copy": "alt",      # "alt" (DVE/ACT alternate) | "dve" (all DVE)
    "out_queue": "act",   # "sp" | "act" | "both" — queue(s) for C's out DMAs
    "w_queue": "sp",      # "sp" | "act" — queue for wk/wv/wo weight loads
    "osb_bufs": 4,        # out-staging buffer depth (4 x 1024-col chunks)
    "ps_out_bufs": 4,     # C-phase PSUM buffer depth
    "c_pair": 1,          # drain two 512-col PSUM chunks per copy inst
    "osb_chunk": 1,       # per-1024-col staging buffers + per-chunk DMA
}


def _mark(nc, label):
    try:
        PHASE_MARKS.append((int(nc.next_id()), label))
    except Exception:
        pass


def _off(dk):
    """Leading fully-masked columns of a causal diagonal block."""
    return dk * 128 if 1 <= dk <= 3 else 0


def _emit_a(nc, d, it, cst, wts, big, xt_pool, tmp_pool, ps_q, ps_sw, ps_v,
            bcst=None):
    """Project Q^T/K^T (rotary fused) and V for seq columns [it*ST, it*ST+ST).

    For it==0, constant/weight DMAs are sequenced between compute emission
    points so nothing clogs the DMA queues ahead of the first matmuls."""
    s0 = it * ST
    qt_sb, kt_sb, v_sb, _ = big
    wq_sb, wk_sb, wv_sb, _ = wts
    cos_sb, sin_sb, bq_sb, bk_sb, bvb_sb, perm_sb = cst

    xtr_v = d["xtr"].rearrange("(ko p) n -> p ko n", p=128)
    xt = xt_pool.tile([128, KSL, ST], BF16, tag="xt", name="xt")
    if it == 0:
        # first small x/wq chunks up front so the first matmul starts after
        # ~250KB of DMA; HWDGE issue is serial (~625ns per dma_start) so
        # everything else ships in big chunks behind them
        wqv = d["wq"].rearrange("(ko p) i -> p ko i", p=128)
        first = True
        for ks in (slice(0, 1), slice(1, 3), slice(3, 7), slice(7, 11),
                   slice(11, 16)):
            nc.sync.dma_start(xt[:, ks, :], xtr_v[:, ks, s0:s0 + ST])
            nc.sync.dma_start(wq_sb[:, ks, :], wqv[:, ks, :])
            if first:
                # small epilogue constants ride just behind the first chunk
                nc.sync.dma_start(bq_sb[:], d["bq"][:])
                nc.sync.dma_start(bk_sb[:], d["bk"][:])
                nc.sync.dma_start(perm_sb[:], d["perm"][:])
                first = False
        # needed by the first delayed epilogue (~10us in), ahead of wk/wv
        nc.sync.dma_start(cos_sb[:], d["cos_t"][:])
        nc.sync.dma_start(sin_sb[:], d["sin_t"][:])
    else:
        for ka in range(2):
            ks = slice(8 * ka, 8 * (ka + 1))
            nc.sync.dma_start(xt[:, ks, :], xtr_v[:, ks, s0:s0 + ST])

    # delayed epilogues: rotary runs two m-groups behind the projection
    # matmuls so PE never waits on the DVE chain feeding the perm matmul
    epi_q = []

    def _epilogue(pq, b_sb, dst, m):
        qtmp = tmp_pool.tile([128, ST], BF16, tag="qtmp")
        nc.vector.tensor_scalar_add(qtmp[:], pq[:], b_sb[:, m:m + 1])
        psw = ps_sw.tile([128, ST], F32, tag="psw")
        nc.tensor.matmul(psw[:], perm_sb[:], qtmp[:], start=True, stop=True)
        t1 = tmp_pool.tile([128, ST], BF16, tag="t1")
        nc.vector.tensor_mul(t1[:], qtmp[:], cos_sb[:, s0:s0 + ST])
        t2 = tmp_pool.tile([128, ST], BF16, tag="t2")
        nc.vector.tensor_mul(t2[:], psw[:], sin_sb[:, s0:s0 + ST])
        nc.vector.tensor_add(dst[:, m, s0:s0 + ST], t1[:], t2[:])

    for wi, (w_sb, b_sb, dst) in enumerate(((wq_sb, bq_sb, qt_sb),
                                            (wk_sb, bk_sb, kt_sb))):
        if it == 0 and wi == 1:
            wkv = d["wk"].rearrange("(ko p) i -> p ko i", p=128)
            wvv = d["wv"].rearrange("(ko p) i -> p ko i", p=128)
            w_eng = nc.scalar if CONFIG["w_queue"] == "act" else nc.sync
            for ka in range(2):
                ks = slice(8 * ka, 8 * (ka + 1))
                w_eng.dma_start(wk_sb[:, ks, :], wkv[:, ks, :])
                w_eng.dma_start(wv_sb[:, ks, :], wvv[:, ks, :])
            if bcst is not None:
                nc.sync.dma_start(bcst[0][:], d["mask"][:])
                nc.sync.dma_start(bcst[1][:], d["ones"][:])
        for m in range(HPC):
            pq = ps_q.tile([128, ST], F32, tag="pq")
            for k in range(KSL):
                nc.tensor.matmul(
                    pq[:], w_sb[:, k, m * 128:(m + 1) * 128], xt[:, k, :],
                    start=(k == 0), stop=(k == KSL - 1))
            epi_q.append((pq, b_sb, dst, m))
            while len(epi_q) > 1:
                _epilogue(*epi_q.pop(0))

    if it == 0:
        nc.sync.dma_start(bvb_sb[:], d["bvb"][:])
    for sb in range(ST // 128):
        pv = ps_v.tile([128, INNER_C], F32, tag="pv")
        for k in range(KSL):
            nc.tensor.matmul(
                pv[:], xt[:, k, sb * 128:(sb + 1) * 128], wv_sb[:, k, :],
                start=(k == 0), stop=(k == KSL - 1))
        if epi_q:
            _epilogue(*epi_q.pop(0))
        nc.vector.tensor_add(v_sb[:, it * (ST // 128) + sb, :], pv[:], bvb_sb[:])
    for item in epi_q:
        _epilogue(*item)
    epi_q.clear()


def _emit_s_pair(nc, it, jb0, h, cst, big, ps_pool, pt_pool):
    """S matmuls + exp (+ causal mask) for key-block pair (jb0, jb0+1) of
    head h in query block it. Returns a flush-queue entry with po/pr unset."""
    qt_sb, kt_sb = big[0], big[1]
    mask_sb = cst[0]
    i0 = it * ST
    dk0 = jb0 - 4 * it
    o0, o1 = _off(dk0), _off(dk0 + 1)
    ps_pair = ps_pool.tile([128, 2, ST], F32, tag="ps")
    # block 1 also computes from o0 (its [o0:o1) is causally masked to
    # zero below) so the paired exp never reads unwritten PSUM
    nc.tensor.matmul(
        ps_pair[:, 0, o0:], kt_sb[:, h, jb0 * 128:(jb0 + 1) * 128],
        qt_sb[:, h, i0 + o0:i0 + ST], start=True, stop=True)
    nc.tensor.matmul(
        ps_pair[:, 1, o0:], kt_sb[:, h, (jb0 + 1) * 128:(jb0 + 2) * 128],
        qt_sb[:, h, i0 + o0:i0 + ST], start=True, stop=True)
    p_t = pt_pool.tile([128, 2, ST], BF16, tag="p")
    nc.scalar.activation(p_t[:, :, o0:], ps_pair[:, :, o0:], EXP)
    if dk0 >= 0:
        nc.vector.tensor_mul(
            p_t[:, :, o0:], p_t[:, :, o0:], mask_sb[:, dk0:dk0 + 2, o0:])
    return (jb0, o0, o1, p_t, None, None, 0)


def _emit_b(nc, it, cst, big, pt_pool, rec_pool, ps_s, ps_o, ps_r):
    """Causal attention for query block it: S^T = K^T.T @ Q^T per 128-key
    block (paired into 2-bank PSUM tiles for one exp each), exp on ACT,
    diagonal masks on DVE, O^T/rowsum accumulation on PE, normalize.

    The O/rowsum flush queue is software-pipelined across the head loop so
    PE never drains waiting for a fresh head's first exp."""
    qt_sb, kt_sb, v_sb, ot_sb = big
    mask_sb, ones_sb = cst
    i0 = it * ST
    njb = 4 * it + 4

    def _flush(jb0, o0, o1, p_t, po_t, pr_t, h):
        # O/R interleaved so consecutive matmuls always ALTERNATE stationary
        # operands (v, ones, v, ones): same-stationary back-to-back MMs
        # measure 232ns vs 130ns alternating (weight-buffer ping-pong),
        # so v,v,ones,ones would hit the slow path on the ones pair.
        # Each PSUM region still accumulates in jb order with its own
        # start/stop flags.
        last = jb0 + 1 == njb - 1
        nc.tensor.matmul(
            po_t[:, o0:], v_sb[:, jb0, h * 128:(h + 1) * 128], p_t[:, 0, o0:],
            start=(jb0 == 0), stop=False)
        nc.tensor.matmul(
            pr_t[:, o0:], ones_sb[:], p_t[:, 0, o0:],
            start=(jb0 == 0), stop=False)
        nc.tensor.matmul(
            po_t[:, o1:], v_sb[:, jb0 + 1, h * 128:(h + 1) * 128],
            p_t[:, 1, o1:],
            start=False, stop=last)
        nc.tensor.matmul(
            pr_t[:, o1:], ones_sb[:], p_t[:, 1, o1:],
            start=False, stop=last)
        if last:
            # approx reciprocal (~18 bits) is plenty for softmax denominators
            # and ~5x cheaper on DVE than InstReciprocal
            rec = rec_pool.tile([128, ST], F32, tag="rec")
            nc.vector.reciprocal_approx_fast(rec[:], pr_t[:])
            nc.vector.tensor_mul(ot_sb[:, h, i0:i0 + ST], po_t[:], rec[:])

    pending = []
    for h in range(HPC):
        po_t = ps_o.tile([128, ST], F32, tag="po")
        pr_t = ps_r.tile([128, ST], F32, tag="pr")
        for pri in range(njb // 2):
            jb0 = 2 * pri
            entry = _emit_s_pair(nc, it, jb0, h, cst, big, ps_s, pt_pool)
            pending.append(entry[:4] + (po_t, pr_t, h))
            while len(pending) > 2:
                _flush(*pending.pop(0))
    for item in pending:
        _flush(*item)


def _emit_c(nc, d, it, big, wts, osb_pool, ps_out):
    """Output projection for the 4 seq row-chunks of query block it."""
    ot_sb = big[3]
    wo_sb = wts[3]
    out_d = d["out"]
    last = it == NIT - 1
    for so in range(it * 4, it * 4 + 4):
        if CONFIG["out_queue"] == "both":
            dma_eng = nc.scalar if so % 2 else nc.sync
        else:
            dma_eng = nc.scalar if CONFIG["out_queue"] == "act" else nc.sync
        chunked = CONFIG.get("c_pair") and CONFIG.get("osb_chunk")
        osb = None if chunked else osb_pool.tile([128, D], BF16, tag="osb")
        split = last and so >= it * 4 + 2
        if CONFIG.get("c_pair"):
            # two matmul chains fill halves of one 2-bank tile; one paired
            # copy drains both (halves the per-instruction copy overhead)
            for np_ in range(D // 1024):
                pout = ps_out.tile([128, 2, 512], F32, tag="poutp")
                for j in range(2):
                    for hh in range(HPC):
                        nc.tensor.matmul(
                            pout[:, j, :],
                            ot_sb[:, hh, so * 128:(so + 1) * 128],
                            wo_sb[:, hh, (2 * np_ + j) * 512:
                                  (2 * np_ + j + 1) * 512],
                            start=(hh == 0), stop=(hh == HPC - 1))
                if chunked:
                    # per-1024 staging buffers + per-chunk DMA: same SBUF
                    # footprint as 2 full-row buffers but 2x finer recycling,
                    # so a copy never waits on a whole row-chunk's DMA drain
                    osbc = osb_pool.tile([128, 1024], BF16, tag="osbc",
                                         name="osbc")
                    dst = osbc[:]
                else:
                    dst = osb[:, np_ * 1024:(np_ + 1) * 1024]
                if CONFIG["c_copy"] == "alt" and np_ % 2 == 1:
                    nc.scalar.activation(dst, pout[:],
                                         mybir.ActivationFunctionType.Copy)
                else:
                    nc.vector.tensor_copy(dst, pout[:])
                if chunked or split or CONFIG.get("out_grain") == 1024:
                    dma_eng.dma_start(
                        out_d[so * 128:(so + 1) * 128,
                              np_ * 1024:(np_ + 1) * 1024], dst)
        else:
            for nt in range(D // 512):
                pout = ps_out.tile([128, 512], F32, tag="pout")
                for hh in range(HPC):
                    nc.tensor.matmul(
                        pout[:], ot_sb[:, hh, so * 128:(so + 1) * 128],
                        wo_sb[:, hh, nt * 512:(nt + 1) * 512],
                        start=(hh == 0), stop=(hh == HPC - 1))
                # alternate the PSUM drain between DVE and ACT so neither
                # becomes the copy bottleneck behind the matmul chains
                if CONFIG["c_copy"] == "alt" and nt % 2 == 1:
                    nc.scalar.activation(osb[:, nt * 512:(nt + 1) * 512],
                                         pout[:],
                                         mybir.ActivationFunctionType.Copy)
                else:
                    nc.vector.tensor_copy(osb[:, nt * 512:(nt + 1) * 512],
                                          pout[:])
                if split:
                    # pipeline the final chunk's copy->DMA to shorten the tail
                    dma_eng.dma_start(
                        out_d[so * 128:(so + 1) * 128,
                              nt * 512:(nt + 1) * 512],
                        osb[:, nt * 512:(nt + 1) * 512])
        if osb is not None and not split and CONFIG.get("out_grain") != 1024:
            dma_eng.dma_start(out_d[so * 128:(so + 1) * 128, :], osb[:])


def _build_program(phases=("A", "B", "C"), repeat=1):
    PHASE_MARKS.clear()
    phases = {{"A1": "A", "A2": "A"}.get(p, p) for p in phases}
    nc = bacc.Bacc("TRN2", target_bir_lowering=False, debug=False,
                   num_devices=NCORES)

    d = {}
    d["xtr"] = nc.dram_tensor("xtr", [D, N], BF16, kind="ExternalInput").ap()
    d["wq"] = nc.dram_tensor("wq", [D, INNER_C], BF16, kind="ExternalInput").ap()
    d["wk"] = nc.dram_tensor("wk", [D, INNER_C], BF16, kind="ExternalInput").ap()
    d["wv"] = nc.dram_tensor("wv", [D, INNER_C], BF16, kind="ExternalInput").ap()
    d["wo"] = nc.dram_tensor("wo", [INNER_C, D], BF16, kind="ExternalInput").ap()
    d["bq"] = nc.dram_tensor("bq", [128, HPC], F32, kind="ExternalInput").ap()
    d["bk"] = nc.dram_tensor("bk", [128, HPC], F32, kind="ExternalInput").ap()
    d["bvb"] = nc.dram_tensor("bvb", [128, INNER_C], F32, kind="ExternalInput").ap()
    d["cos_t"] = nc.dram_tensor("cos_t", [128, N], BF16, kind="ExternalInput").ap()
    d["sin_t"] = nc.dram_tensor("sin_t", [128, N], F32, kind="ExternalInput").ap()
    d["mask"] = nc.dram_tensor("mask", [128, 4, 512], BF16, kind="ExternalInput").ap()
    d["ones"] = nc.dram_tensor("ones", [128, 128], BF16, kind="ExternalInput").ap()
    d["perm"] = nc.dram_tensor("perm", [128, 128], BF16, kind="ExternalInput").ap()
    d["out"] = nc.dram_tensor("out", [N, D], BF16, kind="ExternalOutput").ap()

    with tile.TileContext(nc) as tc:
        for rep in range(repeat):
            sx = f"_{rep}" if rep else ""
            with (
                tc.tile_pool(name="cst" + sx, bufs=1) as cst_pool,
                tc.tile_pool(name="wts" + sx, bufs=1) as wts_pool,
                tc.tile_pool(name="big" + sx, bufs=1) as big_pool,
                tc.tile_pool(name="xt" + sx, bufs=2) as xt_pool,
                tc.tile_pool(name="tmp" + sx, bufs=2) as tmp_pool,
                tc.tile_pool(name="pt" + sx, bufs=5) as pt_pool,
                tc.tile_pool(name="rec" + sx, bufs=2) as rec_pool,
                tc.tile_pool(name="osb" + sx,
                             bufs=CONFIG["osb_bufs"]) as osb_pool,
            ):
                cos_sb = cst_pool.tile([128, N], BF16)
                sin_sb = cst_pool.tile([128, N], F32)
                bq_sb = cst_pool.tile([128, HPC], F32)
                bk_sb = cst_pool.tile([128, HPC], F32)
                bvb_sb = cst_pool.tile([128, INNER_C], F32)
                perm_sb = cst_pool.tile([128, 128], BF16)
                mask_sb = cst_pool.tile([128, 4, 512], BF16)
                ones_sb = cst_pool.tile([128, 128], BF16)
                if "A" not in phases:
                    # _emit_a normally sequences these between its compute
                    for t, key in ((cos_sb, "cos_t"), (sin_sb, "sin_t"),
                                   (bq_sb, "bq"), (bk_sb, "bk"),
                                   (bvb_sb, "bvb"), (perm_sb, "perm")):
                        nc.sync.dma_start(t[:], d[key][:])

                wq_sb = wts_pool.tile([128, KSL, INNER_C], BF16)
                wk_sb = wts_pool.tile([128, KSL, INNER_C], BF16)
                wv_sb = wts_pool.tile([128, KSL, INNER_C], BF16)
                wo_sb = wts_pool.tile([128, HPC, D], BF16)
                wts = (wq_sb, wk_sb, wv_sb, wo_sb)

                qt_sb = big_pool.tile([128, HPC, N], BF16)
                kt_sb = big_pool.tile([128, HPC, N], BF16)
                v_sb = big_pool.tile([128, NJB, INNER_C], BF16)
                ot_sb = big_pool.tile([128, HPC, N], BF16)
                big = (qt_sb, kt_sb, v_sb, ot_sb)

                if "A" not in phases:
                    nc.gpsimd.memset(qt_sb[:], 0.0)
                    nc.gpsimd.memset(kt_sb[:], 0.0)
                    nc.gpsimd.memset(v_sb[:], 0.0)
                if "B" not in phases:
                    nc.gpsimd.memset(ot_sb[:], 0.0)

                acst = (cos_sb, sin_sb, bq_sb, bk_sb, bvb_sb, perm_sb)
                bcst = (mask_sb, ones_sb)

                def emit_c(it):
                    _mark(nc, f"C{it}{sx}")
                    with tc.tile_pool(name=f"ps_out{it}" + sx,
                                      bufs=CONFIG["ps_out_bufs"],
                                      space="PSUM") as ps_out:
                        _emit_c(nc, d, it, big, wts, osb_pool, ps_out)

                for it in range(NIT):
                    if "A" in phases:
                        _mark(nc, f"A{it}{sx}")
                        with (
                            tc.tile_pool(name=f"ps_q{it}" + sx, bufs=3,
                                         space="PSUM") as ps_q,
                            tc.tile_pool(name=f"ps_sw{it}" + sx, bufs=1,
                                         space="PSUM") as ps_sw,
                            tc.tile_pool(name=f"ps_v{it}" + sx, bufs=2,
                                         space="PSUM") as ps_v,
                        ):
                            _emit_a(nc, d, it, acst, wts, big, xt_pool,
                                    tmp_pool, ps_q, ps_sw, ps_v,
                                    bcst=bcst)
                    if it == 0:
                        if "A" not in phases:
                            nc.sync.dma_start(mask_sb[:], d["mask"][:])
                            nc.sync.dma_start(ones_sb[:], d["ones"][:])
                        if "C" in phases:
                            w_eng2 = (nc.scalar if CONFIG["w_queue"] == "act"
                                      else nc.sync)
                            w_eng2.dma_start(
                                wo_sb[:],
                                d["wo"].rearrange("(h p) n -> p h n", p=128)[:])
                    if "B" in phases:
                        _mark(nc, f"B{it}{sx}")
                        with (
                            tc.tile_pool(name=f"ps_s{it}" + sx, bufs=2,
                                         space="PSUM") as ps_s,
                            tc.tile_pool(name=f"ps_o{it}" + sx, bufs=2,
                                         space="PSUM") as ps_o,
                            tc.tile_pool(name=f"ps_r{it}" + sx, bufs=2,
                                         space="PSUM") as ps_r,
                        ):
                            _emit_b(nc, it, bcst, big, pt_pool, rec_pool,
                                    ps_s, ps_o, ps_r)
                    if "C" in phases:
                        emit_c(it)

    nc.compile()
    return nc


def _host_consts():
    scale = DH ** -0.5
    inv_freq = 1.0 / (10000.0 ** (np.arange(0, DH, 2, dtype=np.float32) / DH))
    seq = np.arange(N, dtype=np.float32)
    freqs = np.einsum('i,j->ij', seq, inv_freq)          # [N, 64]
    pos = np.concatenate((freqs, freqs), axis=-1)        # [N, 128]
    cos_t = np.cos(pos).T.astype(NPBF).copy()            # [128, N] bf16
    sin_t = np.sin(pos).T.astype(np.float32)             # [128, N] f32
    sin_t[:64] *= -1.0                                   # rotate_half sign fold

    perm = np.zeros((128, 128), dtype=np.float32)
    perm[(np.arange(128) + 64) % 128, np.arange(128)] = 1.0

    mask = np.zeros((128, 4, 512), dtype=np.float32)
    jj = np.arange(128)[:, None]
    ii = np.arange(512)[None, :]
    for dk in range(4):
        mask[:, dk, :] = (jj + dk * 128 <= ii)

    ones = np.ones((128, 128), dtype=np.float32)
    return scale, cos_t, sin_t, perm.astype(NPBF), mask.astype(NPBF), \
        ones.astype(NPBF)


def make_in_maps(x, Wq, bq, Wk, bk, Wv, bv, Wo, bo):
    x = np.asarray(x, dtype=np.float32)
    Wq = np.asarray(Wq, dtype=np.float32)
    Wk = np.asarray(Wk, dtype=np.float32)
    Wv = np.asarray(Wv, dtype=np.float32)
    Wo = np.asarray(Wo, dtype=np.float32)
    bq = np.asarray(bq, dtype=np.float32)
    bk = np.asarray(bk, dtype=np.float32)
    bv = np.asarray(bv, dtype=np.float32)

    scale, cos_t, sin_t, perm, mask, ones = _host_consts()

    in_maps = []
    for c in range(NCORES):
        b, g = c // GROUPS, c % GROUPS
        sl = slice(g * INNER_C, (g + 1) * INNER_C)
        in_maps.append({
            "xtr": np.ascontiguousarray(x[b].T).astype(NPBF),
            "wq": np.ascontiguousarray(Wq[:, sl] * scale).astype(NPBF),
            "wk": np.ascontiguousarray(Wk[:, sl]).astype(NPBF),
            "wv": np.ascontiguousarray(Wv[:, sl]).astype(NPBF),
            "wo": np.ascontiguousarray(Wo[sl, :]).astype(NPBF),
            "bq": np.ascontiguousarray((bq[sl] * scale).reshape(HPC, 128).T),
            "bk": np.ascontiguousarray(bk[sl].reshape(HPC, 128).T),
            "bvb": np.ascontiguousarray(np.tile(bv[sl], (128, 1))),
            "cos_t": cos_t,
            "sin_t": sin_t,
            "mask": mask,
            "ones": ones,
            "perm": perm,
        })
    return in_maps


def kernel(x, Wq, bq, Wk, bk, Wv, bv, Wo, bo):
    global LAST_RESULTS
    if "nc" not in _CACHE:
        _CACHE["nc"] = _build_program()
    nc = _CACHE["nc"]

    bo = np.asarray(bo, dtype=np.float32)
    in_maps = make_in_maps(x, Wq, bq, Wk, bk, Wv, bv, Wo, bo)

    LAST_RESULTS = run_bass_kernel_spmd(nc, in_maps, core_ids=list(range(NCORES)))
    results = LAST_RESULTS.results

    out = np.zeros((B, N, D), dtype=np.float32)
    for c in range(NCORES):
        out[c // GROUPS] += results[c]["out"].astype(np.float32)
    out += bo
    return out

